# revision 1
# baseline (speedup 1.0000x reference)
"""Trainium2 Bass kernel for nn_NodeNetwork (GNN message passing + MLP + L2 norm).

Data-parallel over the node dimension: 500000 nodes sharded as 62500/core
across 8 NeuronCores; MLP weights replicated. Per core, nodes are processed
in 125 macro-tiles of 500 nodes. Each SBUF partition holds S=4 consecutive
nodes so every DMA descriptor covers 4 contiguous DRAM rows (32KB for the
message tensor, 2KB for features/global/out), and DMA issue is spread over
the sync HWDGE, scalar HWDGE and gpsimd SWDGE queues to parallelize
descriptor generation.
"""

import numpy as np

F = 128
DEG = 16
H1 = 256
H2 = 256
OUT = 128
N_CORES = 8

G = 125          # partitions per macro-tile
S = 4            # consecutive nodes per partition
MACRO = G * S    # 500 nodes per macro-tile
MROW = DEG * F   # 2048 f32 per message row

_NC_CACHE = {}


def build(n_nodes, n_cores=N_CORES, mode="full"):
    import concourse.bacc as bacc
    import concourse.mybir as mybir
    import concourse.tile as tile
    import concourse.masks as masks
    from contextlib import ExitStack

    f32 = mybir.dt.float32
    AX = mybir.AxisListType
    ALU = mybir.AluOpType
    ACTF = mybir.ActivationFunctionType

    assert n_nodes % MACRO == 0
    n_macros = n_nodes // MACRO

    nc = bacc.Bacc(
        "TRN2", target_bir_lowering=False, debug=False, num_devices=n_cores
    )
    if mode.startswith("mini"):
        w1_d = nc.dram_tensor("W1", [3 * F, H1], f32, kind="ExternalInput").ap()
        if mode == "minib":
            nc.dram_tensor("message", [n_nodes, MROW], f32, kind="ExternalInput")
        wb_aps = {}
        if mode in ("minit", "miniw", "minibias", "miniid"):
            wb_aps["b1"] = nc.dram_tensor("b1", [H1], f32, kind="ExternalInput").ap()
            wb_aps["W2"] = nc.dram_tensor("W2", [H1, H2], f32, kind="ExternalInput").ap()
            wb_aps["b2"] = nc.dram_tensor("b2", [H2], f32, kind="ExternalInput").ap()
            wb_aps["W3"] = nc.dram_tensor("W3", [H2, OUT], f32, kind="ExternalInput").ap()
            wb_aps["b3"] = nc.dram_tensor("b3", [OUT], f32, kind="ExternalInput").ap()
        out_d = nc.dram_tensor("out", [n_nodes, OUT], f32, kind="ExternalOutput").ap()
        with tile.TileContext(nc) as tc, ExitStack() as ctx:
            cpool = ctx.enter_context(tc.tile_pool(name="consts", bufs=1))
            opool = ctx.enter_context(tc.tile_pool(name="outp", bufs=1))
            w1sb = cpool.tile([128, 3 * H1], f32, tag="w1")
            nc.sync.dma_start(w1sb[:], w1_d.rearrange("(k p) m -> p k m", p=128))
            if mode == "miniw":
                w2sb = cpool.tile([128, 2 * H2], f32, tag="w2")
                nc.sync.dma_start(w2sb[:], wb_aps["W2"].rearrange("(k p) m -> p k m", p=128))
                w3sb = cpool.tile([128, 2 * OUT], f32, tag="w3")
                nc.sync.dma_start(w3sb[:], wb_aps["W3"].rearrange("(k p) m -> p k m", p=128))
            if mode == "minibias":
                b1sb = cpool.tile([128, 2], f32, tag="b1")
                nc.sync.dma_start(b1sb[:], wb_aps["b1"].rearrange("(m p) -> p m", p=128))
                b2sb = cpool.tile([128, 2], f32, tag="b2")
                nc.sync.dma_start(b2sb[:], wb_aps["b2"].rearrange("(m p) -> p m", p=128))
                b3sb = cpool.tile([128, 1], f32, tag="b3")
                nc.sync.dma_start(b3sb[:], wb_aps["b3"].rearrange("(m p) -> p m", p=128))
            if mode == "miniid":
                ident = cpool.tile([128, 128], f32, tag="ident")
                masks.make_identity(nc, ident[:])
            z = opool.tile([128, S * F], f32, tag="outsb")
            nc.gpsimd.memset(z[:], 0.25)
            nc.scalar.dma_start(
                out_d[0:MACRO].rearrange("(p s) f -> p (s f)", s=S), z[:G]
            )
        nc.compile()
        return nc
    bf16 = mybir.dt.bfloat16
    bf_all = mode.startswith("optc")
    msg_bf16 = mode.startswith(("optb", "optc"))
    mdt = bf16 if msg_bf16 else f32
    sdt = bf16 if bf_all else f32
    msg_d = nc.dram_tensor("message", [n_nodes, MROW], mdt, kind="ExternalInput").ap()
    feat_d = nc.dram_tensor("features", [n_nodes, F], sdt, kind="ExternalInput").ap()
    glob_d = nc.dram_tensor(
        "global_features", [n_nodes, F], sdt, kind="ExternalInput"
    ).ap()
    w1_d = nc.dram_tensor("W1", [3 * F, H1], f32, kind="ExternalInput").ap()
    b1_d = nc.dram_tensor("b1", [H1], f32, kind="ExternalInput").ap()
    w2_d = nc.dram_tensor("W2", [H1, H2], f32, kind="ExternalInput").ap()
    b2_d = nc.dram_tensor("b2", [H2], f32, kind="ExternalInput").ap()
    w3_d = nc.dram_tensor("W3", [H2, OUT], f32, kind="ExternalInput").ap()
    b3_d = nc.dram_tensor("b3", [OUT], f32, kind="ExternalInput").ap()
    out_d = nc.dram_tensor("out", [n_nodes, OUT], sdt, kind="ExternalOutput").ap()

    mbufs = 3
    if mode.endswith(("4", "6")):
        mbufs = int(mode[-1])
        mode = mode[:-1]
    with tile.TileContext(nc) as tc, ExitStack() as ctx:
        cpool = ctx.enter_context(tc.tile_pool(name="consts", bufs=1))
        mpool = ctx.enter_context(tc.tile_pool(name="msg", bufs=mbufs))
        ipool = ctx.enter_context(tc.tile_pool(name="inputs", bufs=3))
        xpool = ctx.enter_context(tc.tile_pool(name="xside", bufs=3))
        xtpool = ctx.enter_context(tc.tile_pool(name="xt", bufs=4))
        hpool = ctx.enter_context(tc.tile_pool(name="hid", bufs=2))
        npool = ctx.enter_context(tc.tile_pool(name="norm", bufs=2))
        opool = ctx.enter_context(tc.tile_pool(name="outp", bufs=3))
        ps_xt = ctx.enter_context(tc.tile_pool(name="ps_xt", bufs=2, space="PSUM"))
        ps_mm = ctx.enter_context(tc.tile_pool(name="ps_mm", bufs=3, space="PSUM"))
        ps_out = ctx.enter_context(tc.tile_pool(name="ps_out", bufs=2, space="PSUM"))

        # --- constants ---
        ident = cpool.tile([128, 128], f32, tag="ident")
        masks.make_identity(nc, ident[:])
        w1sb = cpool.tile([128, 3 * H1], f32, tag="w1")  # [p, (k=3, m*128+c=256)]
        nc.sync.dma_start(w1sb[:], w1_d.rearrange("(k p) m -> p k m", p=128))
        w2sb = cpool.tile([128, 2 * H2], f32, tag="w2")
        nc.sync.dma_start(w2sb[:], w2_d.rearrange("(k p) m -> p k m", p=128))
        w3sb = cpool.tile([128, 2 * OUT], f32, tag="w3")
        nc.sync.dma_start(w3sb[:], w3_d.rearrange("(k p) m -> p k m", p=128))
        b1sb = cpool.tile([128, 2], f32, tag="b1")
        nc.sync.dma_start(b1sb[:], b1_d.rearrange("(m p) -> p m", p=128))
        b2sb = cpool.tile([128, 2], f32, tag="b2")
        nc.sync.dma_start(b2sb[:], b2_d.rearrange("(m p) -> p m", p=128))
        b3sb = cpool.tile([128, 1], f32, tag="b3")
        nc.sync.dma_start(b3sb[:], b3_d.rearrange("(m p) -> p m", p=128))

        if mode == "empty":
            n_macros = 0
        if mode == "noin":
            z = opool.tile([128, S * F], f32, tag="outsb")
            nc.gpsimd.memset(z[:], 0.25)
            for mi in range(n_macros):
                r0 = mi * MACRO
                nc.scalar.dma_start(
                    out_d[r0 : r0 + MACRO].rearrange("(p s) f -> p (s f)", s=S),
                    z[:G],
                )
            n_macros = 0

        GH = 63  # partition split point for the two message-load queues

        # queue assignment per mode: (msg_split, feat_eng, glob_eng, out_eng)
        qplan = {
            "full": (True, nc.gpsimd, nc.gpsimd, nc.gpsimd),
            "nogp": (True, nc.sync, nc.scalar, nc.scalar),
            "synconly": (False, nc.sync, nc.sync, nc.scalar),
            "opt1": (False, nc.sync, nc.sync, nc.scalar),
            "opt2": (False, nc.scalar, nc.scalar, nc.scalar),
            "optalt": ("alt", nc.sync, nc.scalar, nc.scalar),
            "optb": ("alt", nc.sync, nc.scalar, nc.scalar),
            "optgp": ("alt", nc.gpsimd, nc.gpsimd, nc.gpsimd),
            "msgonly": (True, None, None, nc.scalar),
            "msgpure": (False, None, None, nc.scalar),
            "msgpure2": (True, None, None, nc.scalar),
            "msgalt": ("alt", None, None, nc.scalar),
        }
        msg_split, feat_eng, glob_eng, out_eng = qplan.get(
            mode, qplan["full"]
        )
        # modes with the DVE per-partition-scalar normalize moved to Act
        act_norm = mode.startswith("opt")

        for mi in range(n_macros):
            r0 = mi * MACRO
            # --- loads: 4 consecutive DRAM rows per partition ---
            msgt = mpool.tile([128, S * MROW], mdt, tag="msgt")
            if msg_split == "alt":
                eng = nc.sync if mi % 2 == 0 else nc.scalar
                eng.dma_start(
                    msgt[:G],
                    msg_d[r0 : r0 + MACRO].rearrange("(p s) j -> p (s j)", s=S),
                )
            elif msg_split:
                nc.sync.dma_start(
                    msgt[:GH],
                    msg_d[r0 : r0 + GH * S].rearrange("(p s) j -> p (s j)", s=S),
                )
                nc.scalar.dma_start(
                    msgt[GH:G],
                    msg_d[r0 + GH * S : r0 + MACRO].rearrange(
                        "(p s) j -> p (s j)", s=S
                    ),
                )
            else:
                nc.sync.dma_start(
                    msgt[:G],
                    msg_d[r0 : r0 + MACRO].rearrange("(p s) j -> p (s j)", s=S),
                )
            if mode in ("msgpure", "msgpure2", "msgalt"):
                if mi == n_macros - 1:
                    z = opool.tile([128, S * F], f32, tag="outsb")
                    nc.vector.tensor_copy(z[:G], msgt[:G, : S * F])
                    out_eng.dma_start(
                        out_d[r0 : r0 + MACRO].rearrange("(p s) f -> p (s f)", s=S),
                        z[:G],
                    )
                continue
            if mode == "msgonly":
                z = opool.tile([128, S * F], f32, tag="outsb")
                nc.vector.tensor_copy(z[:G], msgt[:G, : S * F])
                out_eng.dma_start(
                    out_d[r0 : r0 + MACRO].rearrange("(p s) f -> p (s f)", s=S),
                    z[:G],
                )
                continue
            featt = ipool.tile([128, S * F], f32, tag="featt")
            feat_eng.dma_start(
                featt[:G],
                feat_d[r0 : r0 + MACRO].rearrange("(p s) f -> p (s f)", s=S),
            )
            globt = ipool.tile([128, S * F], f32, tag="globt")
            glob_eng.dma_start(
                globt[:G],
                glob_d[r0 : r0 + MACRO].rearrange("(p s) f -> p (s f)", s=S),
            )

            # --- mailbox sum over DEG (node-major) ---
            xagg = xpool.tile([128, S * F], f32, tag="xagg")
            for s in range(S):
                nc.vector.tensor_reduce(
                    xagg[:G, s * F : (s + 1) * F],
                    msgt[:G, s * MROW : (s + 1) * MROW].rearrange(
                        "p (d f) -> p f d", f=F
                    ),
                    axis=AX.X,
                    op=ALU.add,
                )

            # --- transpose x pieces to feature-major [128, 500] ---
            xts = []
            for src in (xagg, featt, globt):
                pxt = ps_xt.tile([128, 512], f32, tag="pxt")
                for s in range(S):
                    nc.tensor.transpose(
                        pxt[:, s * G : (s + 1) * G],
                        src[:G, s * F : (s + 1) * F],
                        ident[:G, :G],
                    )
                xt = xtpool.tile([128, MACRO], f32, tag="xt")
                nc.scalar.copy(xt[:], pxt[:, :MACRO])
                xts.append(xt)

            # --- layer 1: [384 -> 256], relu ---
            h1 = hpool.tile([128, 2 * MACRO], f32, tag="h1")
            for m in range(2):
                pmm = ps_mm.tile([128, MACRO], f32, tag="pmm")
                for k in range(3):
                    nc.tensor.matmul(
                        pmm[:],
                        w1sb[:, k * H1 + m * 128 : k * H1 + (m + 1) * 128],
                        xts[k][:],
                        start=(k == 0),
                        stop=(k == 2),
                    )
                nc.scalar.activation(
                    h1[:, m * MACRO : (m + 1) * MACRO],
                    pmm[:],
                    ACTF.Relu,
                    bias=b1sb[:, m : m + 1],
                )

            # --- layer 2: [256 -> 256], relu ---
            h2 = hpool.tile([128, 2 * MACRO], f32, tag="h2")
            for m in range(2):
                pmm = ps_mm.tile([128, MACRO], f32, tag="pmm")
                for k in range(2):
                    nc.tensor.matmul(
                        pmm[:],
                        w2sb[:, k * H2 + m * 128 : k * H2 + (m + 1) * 128],
                        h1[:, k * MACRO : (k + 1) * MACRO],
                        start=(k == 0),
                        stop=(k == 1),
                    )
                nc.scalar.activation(
                    h2[:, m * MACRO : (m + 1) * MACRO],
                    pmm[:],
                    ACTF.Relu,
                    bias=b2sb[:, m : m + 1],
                )

            # --- layer 3: [256 -> 128], + b3 ---
            pmm = ps_mm.tile([128, MACRO], f32, tag="pmm")
            for k in range(2):
                nc.tensor.matmul(
                    pmm[:],
                    w3sb[:, k * OUT : (k + 1) * OUT],
                    h2[:, k * MACRO : (k + 1) * MACRO],
                    start=(k == 0),
                    stop=(k == 1),
                )
            o3 = hpool.tile([128, MACRO], f32, tag="o3")
            nc.scalar.activation(o3[:], pmm[:], ACTF.Identity, bias=b3sb[:, 0:1])

            # --- transpose back to node-major ---
            pout = ps_out.tile([128, S * F], f32, tag="pout")
            for s in range(S):
                nc.tensor.transpose(
                    pout[:G, s * F : (s + 1) * F],
                    o3[:, s * G : (s + 1) * G],
                    ident[:, :],
                )

            # --- row L2 norm ---
            sq = npool.tile([128, S * F], f32, tag="sq")
            nsq = npool.tile([128, S], f32, tag="nsq")
            for s in range(S):
                nc.scalar.activation(
                    sq[:G, s * F : (s + 1) * F],
                    pout[:G, s * F : (s + 1) * F],
                    ACTF.Square,
                    accum_out=nsq[:G, s : s + 1],
                )
            nv = npool.tile([128, S], f32, tag="nv")
            nc.scalar.activation(nv[:G], nsq[:G], ACTF.Sqrt)
            nve = npool.tile([128, S], f32, tag="nve")
            nc.vector.tensor_scalar_add(nve[:G], nv[:G], 1e-8)
            ri = npool.tile([128, S], f32, tag="ri")
            nc.vector.reciprocal(ri[:G], nve[:G])

            outsb = opool.tile([128, S * F], f32, tag="outsb")
            for s in range(S):
                if act_norm:
                    nc.scalar.activation(
                        outsb[:G, s * F : (s + 1) * F],
                        pout[:G, s * F : (s + 1) * F],
                        ACTF.Copy,
                        scale=ri[:G, s : s + 1],
                    )
                else:
                    nc.vector.tensor_scalar_mul(
                        outsb[:G, s * F : (s + 1) * F],
                        pout[:G, s * F : (s + 1) * F],
                        ri[:G, s : s + 1],
                    )

            # --- store ---
            out_eng.dma_start(
                out_d[r0 : r0 + MACRO].rearrange("(p s) f -> p (s f)", s=S),
                outsb[:G],
            )

    nc.compile()
    return nc


def _get_nc(n_nodes, n_cores, mode="full"):
    key = (n_nodes, n_cores, mode)
    if key not in _NC_CACHE:
        _NC_CACHE[key] = build(n_nodes, n_cores, mode)
    return _NC_CACHE[key]


PROD_MODE = "optb4"


def kernel(message, features, global_features, W1, b1, W2, b2, W3, b3):
    import ml_dtypes
    from concourse.bass_utils import run_bass_kernel_spmd

    n = message.shape[0]
    assert n % N_CORES == 0
    npc = n // N_CORES

    nc = _get_nc(npc, N_CORES, PROD_MODE)

    def shard(a, shape, dtype=np.float32):
        return np.ascontiguousarray(
            np.asarray(a).astype(dtype).reshape((N_CORES,) + shape)
        )

    mdt = ml_dtypes.bfloat16 if PROD_MODE.startswith("optb") else np.float32
    msg = shard(message, (npc, MROW), mdt)
    feat = shard(features, (npc, F))
    glob = shard(global_features, (npc, F))
    w1 = np.ascontiguousarray(np.asarray(W1, np.float32))
    w2 = np.ascontiguousarray(np.asarray(W2, np.float32))
    w3 = np.ascontiguousarray(np.asarray(W3, np.float32))
    bb1 = np.ascontiguousarray(np.asarray(b1, np.float32))
    bb2 = np.ascontiguousarray(np.asarray(b2, np.float32))
    bb3 = np.ascontiguousarray(np.asarray(b3, np.float32))

    in_maps = [
        {
            "message": msg[i],
            "features": feat[i],
            "global_features": glob[i],
            "W1": w1,
            "b1": bb1,
            "W2": w2,
            "b2": bb2,
            "W3": w3,
            "b3": bb3,
        }
        for i in range(N_CORES)
    ]
    res = run_bass_kernel_spmd(nc, in_maps, list(range(N_CORES))).results
    return np.concatenate([res[i]["out"] for i in range(N_CORES)], axis=0)



# revision 7
# speedup vs baseline: 1.4929x; 1.4929x over previous
"""Trainium2 Bass kernel for nn_NodeNetwork (GNN message passing + MLP + L2 norm).

Data-parallel over nodes: 500000 nodes sharded 62500/core across 8 NeuronCores,
MLP weights replicated. All streaming tensors are bf16 on the wire (message,
features, global_features, weights, output) — per-core HBM traffic is
256 + 16 + 16 + 16 = 304 MB against a ~358 GB/s HBM limit (~850 us roofline).

Per core, nodes are processed in macro-tiles of 512 (node = g*128 + p for
g in 0..3, p = SBUF partition). The mailbox sum over DEG=16 is a pairwise
tensor_tensor add tree (4 passes, contiguous bf16 operands) so the DVE runs
in its 2x perf mode — a plain tensor_reduce over a strided axis runs at 1x
and becomes the bottleneck. The MLP runs feature-major: features/globals are
loaded via HWDGE DMA-transpose straight into [F, nodes] layout, the aggregate
is transposed on the PE, and matmuls keep the weights stationary. Outputs are
transposed back on the PE, L2-normalized per node (Square+accum on DVE,
sqrt + scale-copy on ACT), and stored node-major.

The 62500 = 122*512 + 36 remainder is handled by one 48-node tail tile that
re-reads/re-computes 12 overlap nodes but only stores the final 36.
"""

import numpy as np

F = 128
DEG = 16
H1 = 256
H2 = 256
OUT = 128
N_CORES = 8

MROW = DEG * F   # 2048 elems per message row
MACRO = 512      # nodes per macro-tile
NG = 4           # node groups of 128 per macro-tile
TAILR = 48       # tail tile row-read width (multiple of 16 for DMA transpose)

_NC_CACHE = {}


def build_fmaj(n_nodes, n_cores=N_CORES, msg_bufs=3):
    import concourse.bacc as bacc
    import concourse.mybir as mybir
    import concourse.tile as tile
    import concourse.masks as masks
    from contextlib import ExitStack

    f32 = mybir.dt.float32
    bf16 = mybir.dt.bfloat16
    ALU = mybir.AluOpType
    ACTF = mybir.ActivationFunctionType

    nm = n_nodes // MACRO
    tail_n = n_nodes - nm * MACRO
    if tail_n:
        assert n_nodes >= TAILR and tail_n <= TAILR

    nc = bacc.Bacc(
        "TRN2", target_bir_lowering=False, debug=False, num_devices=n_cores
    )
    msg_d = nc.dram_tensor("message", [n_nodes, MROW], bf16, kind="ExternalInput").ap()
    feat_d = nc.dram_tensor("features", [n_nodes, F], bf16, kind="ExternalInput").ap()
    glob_d = nc.dram_tensor(
        "global_features", [n_nodes, F], bf16, kind="ExternalInput"
    ).ap()
    w1_d = nc.dram_tensor("W1", [3 * F, H1], bf16, kind="ExternalInput").ap()
    b1_d = nc.dram_tensor("b1", [H1], f32, kind="ExternalInput").ap()
    w2_d = nc.dram_tensor("W2", [H1, H2], bf16, kind="ExternalInput").ap()
    b2_d = nc.dram_tensor("b2", [H2], f32, kind="ExternalInput").ap()
    w3_d = nc.dram_tensor("W3", [H2, OUT], bf16, kind="ExternalInput").ap()
    b3_d = nc.dram_tensor("b3", [OUT], f32, kind="ExternalInput").ap()
    out_d = nc.dram_tensor("out", [n_nodes, OUT], bf16, kind="ExternalOutput").ap()

    with tile.TileContext(nc) as tc, ExitStack() as ctx:
        cpool = ctx.enter_context(tc.tile_pool(name="consts", bufs=1))
        mpool = ctx.enter_context(tc.tile_pool(name="msg", bufs=msg_bufs))
        tpool = ctx.enter_context(tc.tile_pool(name="tree", bufs=2))
        xpool = ctx.enter_context(tc.tile_pool(name="xmaj", bufs=3))
        hpool = ctx.enter_context(tc.tile_pool(name="hid", bufs=2))
        npool = ctx.enter_context(tc.tile_pool(name="norm", bufs=2))
        opool = ctx.enter_context(tc.tile_pool(name="outp", bufs=3))
        ps_xt = ctx.enter_context(tc.tile_pool(name="ps_xt", bufs=2, space="PSUM"))
        ps_mm = ctx.enter_context(tc.tile_pool(name="ps_mm", bufs=3, space="PSUM"))
        ps_out = ctx.enter_context(tc.tile_pool(name="ps_out", bufs=2, space="PSUM"))

        # --- constants ---
        ident = cpool.tile([128, 128], bf16, tag="ident")
        masks.make_identity(nc, ident[:])
        w1sb = cpool.tile([128, 3 * H1], bf16, tag="w1")  # [f, (k, m)]
        nc.sync.dma_start(w1sb[:], w1_d.rearrange("(k p) m -> p k m", p=128))
        w2sb = cpool.tile([128, 2 * H2], bf16, tag="w2")
        nc.sync.dma_start(w2sb[:], w2_d.rearrange("(k p) m -> p k m", p=128))
        w3sb = cpool.tile([128, 2 * OUT], bf16, tag="w3")
        nc.sync.dma_start(w3sb[:], w3_d.rearrange("(k p) m -> p k m", p=128))
        b1sb = cpool.tile([128, 2], f32, tag="b1")
        nc.sync.dma_start(b1sb[:], b1_d.rearrange("(m p) -> p m", p=128))
        b2sb = cpool.tile([128, 2], f32, tag="b2")
        nc.sync.dma_start(b2sb[:], b2_d.rearrange("(m p) -> p m", p=128))
        b3sb = cpool.tile([128, 1], f32, tag="b3")
        nc.sync.dma_start(b3sb[:], b3_d.rearrange("(m p) -> p m", p=128))

        def emit_tile(r0, P, groups, store_lo):
            nodes = groups * P

            # --- message load: node g*P+p lands on partition p, group g ---
            msgt = mpool.tile([128, NG * MROW], bf16, tag="msgt")
            nc.sync.dma_start(
                msgt[:P, : groups * MROW].rearrange("p (g j) -> p g j", g=groups),
                msg_d[r0 : r0 + nodes].rearrange("(g p) j -> p g j", g=groups),
            )

            # --- feature-major side inputs via DMA transpose ---
            featT = xpool.tile([128, MACRO], bf16, tag="featT")
            nc.sync.dma_start(
                featT[:, :nodes], feat_d[r0 : r0 + nodes], transpose=True
            )
            globT = xpool.tile([128, MACRO], bf16, tag="globT")
            nc.sync.dma_start(
                globT[:, :nodes], glob_d[r0 : r0 + nodes], transpose=True
            )

            # --- mailbox sum over DEG=16: pairwise add tree (DVE 2x mode) ---
            widths = [1024, 512, 256, 128]
            src = msgt
            for li, w in enumerate(widths):
                dst_cols = groups * w
                if li < len(widths) - 1:
                    dst = tpool.tile([128, NG * w], bf16, tag=f"t{li}")
                else:
                    dst = tpool.tile([128, NG * F], bf16, tag="xagg")
                v = src[:P, : groups * 2 * w].rearrange(
                    "p (g h x) -> p g h x", g=groups, h=2, x=w
                )
                nc.vector.tensor_tensor(
                    dst[:P, :dst_cols].rearrange("p (g x) -> p g x", g=groups, x=w),
                    v[:, :, 0:1, :],
                    v[:, :, 1:2, :],
                    op=ALU.add,
                )
                src = dst
            xagg = src  # [P, (g, F)] bf16, node-major

            # --- transpose aggregate to feature-major [F, nodes] ---
            pxt = ps_xt.tile([128, MACRO], bf16, tag="pxt")
            for g in range(groups):
                nc.tensor.transpose(
                    pxt[:, g * P : (g + 1) * P],
                    xagg[:P, g * F : (g + 1) * F],
                    ident[:P, :P],
                )
            xaggT = xpool.tile([128, MACRO], bf16, tag="xaggT")
            nc.scalar.copy(xaggT[:, :nodes], pxt[:, :nodes])

            # --- MLP, feature-major (weights stationary) ---
            xts = (xaggT, featT, globT)
            h1 = hpool.tile([128, 2 * MACRO], bf16, tag="h1")
            for m in range(2):
                pmm = ps_mm.tile([128, MACRO], f32, tag="pmm")
                for k in range(3):
                    nc.tensor.matmul(
                        pmm[:, :nodes],
                        w1sb[:, k * H1 + m * 128 : k * H1 + (m + 1) * 128],
                        xts[k][:, :nodes],
                        start=(k == 0),
                        stop=(k == 2),
                    )
                nc.scalar.activation(
                    h1[:, m * MACRO : m * MACRO + nodes],
                    pmm[:, :nodes],
                    ACTF.Relu,
                    bias=b1sb[:, m : m + 1],
                )

            h2 = hpool.tile([128, 2 * MACRO], bf16, tag="h2")
            for m in range(2):
                pmm = ps_mm.tile([128, MACRO], f32, tag="pmm")
                for k in range(2):
                    nc.tensor.matmul(
                        pmm[:, :nodes],
                        w2sb[:, k * H2 + m * 128 : k * H2 + (m + 1) * 128],
                        h1[:, k * MACRO : k * MACRO + nodes],
                        start=(k == 0),
                        stop=(k == 1),
                    )
                nc.scalar.activation(
                    h2[:, m * MACRO : m * MACRO + nodes],
                    pmm[:, :nodes],
                    ACTF.Relu,
                    bias=b2sb[:, m : m + 1],
                )

            pmm = ps_mm.tile([128, MACRO], f32, tag="pmm")
            for k in range(2):
                nc.tensor.matmul(
                    pmm[:, :nodes],
                    w3sb[:, k * OUT : (k + 1) * OUT],
                    h2[:, k * MACRO : k * MACRO + nodes],
                    start=(k == 0),
                    stop=(k == 1),
                )
            o3 = hpool.tile([128, MACRO], bf16, tag="o3")
            nc.scalar.activation(
                o3[:, :nodes], pmm[:, :nodes], ACTF.Identity, bias=b3sb[:, 0:1]
            )

            # --- back to node-major ---
            pout = ps_out.tile([128, NG * F], bf16, tag="pout")
            for g in range(groups):
                nc.tensor.transpose(
                    pout[:P, g * F : (g + 1) * F],
                    o3[:, g * P : (g + 1) * P],
                    ident[:, :],
                )

            # --- row L2 norm ---
            sq = npool.tile([128, NG * F], bf16, tag="sq")
            nsq = npool.tile([128, NG], f32, tag="nsq")
            for g in range(groups):
                nc.scalar.activation(
                    sq[:P, g * F : (g + 1) * F],
                    pout[:P, g * F : (g + 1) * F],
                    ACTF.Square,
                    accum_out=nsq[:P, g : g + 1],
                )
            nv = npool.tile([128, NG], f32, tag="nv")
            nc.scalar.activation(nv[:P, :groups], nsq[:P, :groups], ACTF.Sqrt)
            nve = npool.tile([128, NG], f32, tag="nve")
            nc.vector.tensor_scalar_add(nve[:P, :groups], nv[:P, :groups], 1e-8)
            ri = npool.tile([128, NG], f32, tag="ri")
            nc.vector.reciprocal(ri[:P, :groups], nve[:P, :groups])

            outsb = opool.tile([128, NG * F], bf16, tag="outsb")
            for g in range(groups):
                nc.vector.tensor_scalar_mul(
                    outsb[:P, g * F : (g + 1) * F],
                    pout[:P, g * F : (g + 1) * F],
                    ri[:P, g : g + 1],
                )

            # --- store (tail tile stores only its last tail_n nodes) ---
            if store_lo == 0:
                nc.gpsimd.dma_start(
                    out_d[r0 : r0 + nodes].rearrange("(g p) f -> p g f", g=groups),
                    outsb[:P, : groups * F].rearrange("p (g f) -> p g f", g=groups),
                )
            else:
                nc.gpsimd.dma_start(
                    out_d[r0 + store_lo : r0 + nodes],
                    outsb[store_lo:P, :F],
                )

        for mi in range(nm):
            emit_tile(mi * MACRO, 128, NG, 0)
        if tail_n:
            emit_tile(n_nodes - TAILR, TAILR, 1, TAILR - tail_n)

    nc.compile()
    return nc


def _get_nc(n_nodes, n_cores=N_CORES, mode="fmaj"):
    key = (n_nodes, n_cores, mode)
    if key not in _NC_CACHE:
        assert mode == "fmaj"
        _NC_CACHE[key] = build_fmaj(n_nodes, n_cores)
    return _NC_CACHE[key]


PROD_MODE = "fmaj"


def make_in_maps(message, features, global_features, W1, b1, W2, b2, W3, b3):
    """Shard + cast full inputs into the per-core input maps the NEFF expects."""
    import ml_dtypes

    bf = ml_dtypes.bfloat16
    n = message.shape[0]
    assert n % N_CORES == 0
    npc = n // N_CORES

    def shard(a, shape, dtype):
        return np.ascontiguousarray(
            np.asarray(a).astype(dtype).reshape((N_CORES,) + shape)
        )

    msg = shard(message, (npc, MROW), bf)
    feat = shard(features, (npc, F), bf)
    glob = shard(global_features, (npc, F), bf)
    w1 = np.ascontiguousarray(np.asarray(W1).astype(bf))
    w2 = np.ascontiguousarray(np.asarray(W2).astype(bf))
    w3 = np.ascontiguousarray(np.asarray(W3).astype(bf))
    bb1 = np.ascontiguousarray(np.asarray(b1, np.float32))
    bb2 = np.ascontiguousarray(np.asarray(b2, np.float32))
    bb3 = np.ascontiguousarray(np.asarray(b3, np.float32))
    return [
        {
            "message": msg[i],
            "features": feat[i],
            "global_features": glob[i],
            "W1": w1,
            "b1": bb1,
            "W2": w2,
            "b2": bb2,
            "W3": w3,
            "b3": bb3,
        }
        for i in range(N_CORES)
    ], npc


def kernel(message, features, global_features, W1, b1, W2, b2, W3, b3):
    from concourse.bass_utils import run_bass_kernel_spmd

    in_maps, npc = make_in_maps(
        message, features, global_features, W1, b1, W2, b2, W3, b3
    )
    nc = _get_nc(npc, N_CORES, PROD_MODE)
    res = run_bass_kernel_spmd(nc, in_maps, list(range(N_CORES))).results
    return np.concatenate(
        [np.asarray(res[i]["out"]).astype(np.float32) for i in range(N_CORES)], axis=0
    )
